# revision 20
# baseline (speedup 1.0000x reference)
"""Trainium2 8-core attention kernel (B=2, N=2048, D=1024, H=16).

Sharding: core c = 4*b + g handles batch b, query rows [g*512, (g+1)*512),
all 16 heads. Stage 1 computes QKV^T (channel-major) for the row shard in 4
head-group chunks; each chunk's K/V shards AllGather across the batch's
4-core group and later chunks overlap attention. Attention runs in
transposed-score orientation (S^T[k, q]) with no on-chip transposes;
softmax denominators come from a ones-column appended to V; unnormalized
O^T accumulates in PSUM and is normalized with a partition-broadcast
reciprocal; O^T (channel-major) feeds the output projection directly as the
stationary operand. Host only reshapes/casts inputs and concatenates the 8
output slices.
"""

import sys

if "/opt/trn_rl_repo" not in sys.path:
    sys.path.insert(0, "/opt/trn_rl_repo")

import numpy as np
import ml_dtypes

import concourse.bass as bass
import concourse.mybir as mybir
from concourse import bacc, tile
from concourse import bass_utils

FP32 = mybir.dt.float32
BF16 = mybir.dt.bfloat16

B, N, D = 2, 2048, 1024
H, HD = 16, 64
SCALE = HD ** -0.5
NC = 8
GROUPS = [[0, 1, 2, 3], [4, 5, 6, 7]]
NQ = N // 4          # query rows per core (512)
KT = N // 128        # key k-tiles (16)
CT = D // 128        # 128-channel tiles per D (8)
CHUNKS = [1] * 8                     # AllGather chunks, in duos (head pairs)
DUO_ELEMS = 128 * NQ + 2 * NQ * HD   # per-duo: 1 K^T pair + 2 V heads
CH_OFF = [sum(CHUNKS[:i]) for i in range(len(CHUNKS) + 1)]  # duo offsets

_compiled = None


def build():
    from contextlib import ExitStack

    nc = bacc.Bacc("TRN2", target_bir_lowering=False, debug=False, num_devices=NC)

    xT = nc.dram_tensor("xT", [D, NQ], BF16, kind="ExternalInput")
    w_qkv = nc.dram_tensor("w_qkv", [D, 3 * D], BF16, kind="ExternalInput")
    w_proj = nc.dram_tensor("w_proj", [D, D], BF16, kind="ExternalInput")
    b_qk = nc.dram_tensor("b_qk", [128, 16], FP32, kind="ExternalInput")
    b_v = nc.dram_tensor("b_v", [128, D], FP32, kind="ExternalInput")
    b_prj = nc.dram_tensor("b_prj", [128, D], FP32, kind="ExternalInput")
    out = nc.dram_tensor("out", [NQ, D], FP32, kind="ExternalOutput")

    with tile.TileContext(nc) as tc, ExitStack() as ctx:
        if True:
            wqk_pool = ctx.enter_context(tc.tile_pool(name="wqk", bufs=16))
            wv_pool = ctx.enter_context(tc.tile_pool(name="wv", bufs=8))
            wp_pool = ctx.enter_context(tc.tile_pool(name="wp", bufs=8))
            xt_pool = ctx.enter_context(tc.tile_pool(name="xt", bufs=8))
            qt_pool = ctx.enter_context(tc.tile_pool(name="qt", bufs=8))
            bias_pool = ctx.enter_context(tc.tile_pool(name="bias", bufs=3))
            stg_pool = ctx.enter_context(tc.tile_pool(name="stg", bufs=4))
            ktp_pool = ctx.enter_context(tc.tile_pool(name="ktp", bufs=4))
            vsb_pool = ctx.enter_context(tc.tile_pool(name="vsb", bufs=6))
            es_pool = ctx.enter_context(tc.tile_pool(name="es", bufs=4))
            ot_pool = ctx.enter_context(tc.tile_pool(name="ot", bufs=8))
            nrm_pool = ctx.enter_context(tc.tile_pool(name="nrm", bufs=4))
            y_pool = ctx.enter_context(tc.tile_pool(name="yy", bufs=3))
            ps1 = ctx.enter_context(tc.tile_pool(name="ps1", bufs=4, space="PSUM"))
            psS = ctx.enter_context(tc.tile_pool(name="psS", bufs=2, space="PSUM"))
            dram = ctx.enter_context(tc.tile_pool(name="dram", bufs=1, space="DRAM"))

            # ---- x^T first (needed by every stage-1 matmul) ----
            xt = []
            for k in range(CT):
                t = xt_pool.tile([128, NQ], BF16, tag="xt", name=f"xt{k}")
                nc.sync.dma_start(t[:], xT.ap()[k * 128:(k + 1) * 128, :])
                xt.append(t)

            # ---- biases (small; scalar queue) ----
            bqk_sb = bias_pool.tile([128, 16], FP32, tag="bias")
            nc.sync.dma_start(bqk_sb[:], b_qk.ap()[:])
            bv_sb = bias_pool.tile([128, D], FP32, tag="bias")
            nc.sync.dma_start(bv_sb[:], b_v.ap()[:])
            bp_sb = bias_pool.tile([128, D], FP32, tag="bias")
            nc.sync.dma_start(bp_sb[:], b_prj.ap()[:])

            # ---- K / V weight tiles: [128, 1024] per k-tile ----
            wqk_k = []
            wv_t = []
            for k in range(CT):
                t = wqk_pool.tile([128, D], BF16, tag="wqk", name=f"wqkK{k}")
                nc.sync.dma_start(t[:], w_qkv.ap()[k * 128:(k + 1) * 128, D:2 * D])
                wqk_k.append(t)
                t = wv_pool.tile([128, D], BF16, tag="wv", name=f"wv{k}")
                nc.sync.dma_start(t[:], w_qkv.ap()[k * 128:(k + 1) * 128, 2 * D:3 * D])
                wv_t.append(t)

            # ---- DRAM bounce + AG buffers, chunk-major ----
            # per chunk (n duos): [n K^T pairs (n*128*512)] then V as
            # [4 m-blocks, 128 p, 2n heads, 64 e] (contiguous stores)
            TOT = CH_OFF[-1] * DUO_ELEMS
            kv_in = dram.tile([TOT], BF16, tag="kvin")
            kv_ag = dram.tile([4 * TOT], BF16, tag="kvag")

            # ---- stage 1 per chunk: K^T pairs + V heads, then AG ----
            for c, nduo in enumerate(CHUNKS):
                base = CH_OFF[c] * DUO_ELEMS
                d0 = CH_OFF[c]
                ksz = nduo * 128 * NQ
                kin = kv_in[base:base + ksz].rearrange("(p q) -> p q", q=NQ)
                vin = kv_in[base + ksz:base + nduo * DUO_ELEMS].rearrange(
                    "(m p h e) -> m p h e", m=4, p=128, h=2 * nduo, e=HD
                )
                for tt in range(nduo):
                    d = d0 + tt
                    ps = ps1.tile([128, NQ], FP32, tag="acc", name=f"psK{c}{tt}")
                    for k in range(CT):
                        nc.tensor.matmul(
                            ps[:], wqk_k[k][:, d * 128:(d + 1) * 128], xt[k][:],
                            start=(k == 0), stop=(k == CT - 1),
                        )
                    sb = stg_pool.tile([128, NQ], BF16, tag="stg", name=f"ksb{c}{tt}")
                    nc.vector.tensor_scalar_add(
                        sb[:], ps[:], bqk_sb[:, 8 + d:9 + d]
                    )
                    nc.sync.dma_start(kin[tt * 128:(tt + 1) * 128, :], sb[:])
                for m in range(NQ // 128):
                    ps = ps1.tile([128, 128 * nduo], FP32, tag="acc",
                                  name=f"psV{c}{m}")
                    for k in range(CT):
                        nc.tensor.matmul(
                            ps[:], xt[k][:, m * 128:(m + 1) * 128],
                            wv_t[k][:, d0 * 128:(d0 + nduo) * 128],
                            start=(k == 0), stop=(k == CT - 1),
                        )
                    sb = stg_pool.tile([128, 128 * nduo], BF16, tag="stg",
                                       name=f"vsb{c}{m}")
                    nc.vector.scalar_tensor_tensor(
                        sb[:], ps[:], 0.0,
                        bv_sb[:, d0 * 128:(d0 + nduo) * 128],
                        op0=mybir.AluOpType.bypass, op1=mybir.AluOpType.add,
                    )
                    nc.sync.dma_start(
                        vin[m], sb[:].rearrange("p (h e) -> p h e", e=HD)
                    )
                nc.gpsimd.collective_compute(
                    "AllGather", mybir.AluOpType.bypass, replica_groups=GROUPS,
                    ins=[kv_in[base:base + nduo * DUO_ELEMS].opt()],
                    outs=[kv_ag[4 * base:4 * (base + nduo * DUO_ELEMS)].opt()],
                )

            # ---- Q-half of W_qkv + W_proj (overlap the AGs) ----
            wqk_q = []
            for k in range(CT):
                t = wqk_pool.tile([128, D], BF16, tag="wqk", name=f"wqkQ{k}")
                nc.sync.dma_start(t[:], w_qkv.ap()[k * 128:(k + 1) * 128, 0:D])
                wqk_q.append(t)
            wp = []
            for k in range(CT):
                t = wp_pool.tile([128, D], BF16, tag="wp", name=f"wp{k}")
                nc.sync.dma_start(t[:], w_proj.ap()[k * 128:(k + 1) * 128, :])
                wp.append(t)

            # ---- stage 1: Q^T (channels 0:1024), stays in SBUF ----
            qt = []
            for t in range(CT):
                ps = ps1.tile([128, NQ], FP32, tag="acc", name=f"psQ{t}")
                for k in range(CT):
                    nc.tensor.matmul(
                        ps[:], wqk_q[k][:, t * 128:(t + 1) * 128], xt[k][:],
                        start=(k == 0), stop=(k == CT - 1),
                    )
                sb = qt_pool.tile([128, NQ], BF16, tag="qt", name=f"qt{t}")
                nc.vector.tensor_scalar_add(sb[:], ps[:], bqk_sb[:, t:t + 1])
                qt.append(sb)

            # ---- attention: 8 duos (head pairs), software-pipelined ----
            # per-chunk gathered block (4*CH_ELEMS elems):
            #   rank-major: [4 ranks][2 pairs, 128, 512 | 4 heads, 512, 64]
            duo_chunk = {}
            for c, nduo in enumerate(CHUNKS):
                for tt in range(nduo):
                    duo_chunk[CH_OFF[c] + tt] = (c, tt)
            ot = []
            pend = []          # (o_acc pair, duo idx) awaiting normalization

            def drain_oacc(o_acc, d):
                # fast psum -> sbuf copy so the o_acc slots free immediately
                ocs = []
                for j in range(2):
                    oc = nrm_pool.tile([HD + 1, NQ], FP32, tag="oc",
                                       name=f"oc{d}_{j}")
                    nc.vector.tensor_copy(oc[:], o_acc[j][0:HD + 1, :])
                    ocs.append(oc)
                return ocs

            def normalize(ocs, d):
                otd = ot_pool.tile([128, NQ], BF16, tag="ot", name=f"ot{d}")
                for j in range(2):
                    rr = nrm_pool.tile([1, NQ], FP32, tag="rr", name=f"rr{d}_{j}")
                    nc.vector.reciprocal(rr[:], ocs[j][HD:HD + 1, :])
                    rb = nrm_pool.tile([HD, NQ], FP32, tag="rb", name=f"rb{d}_{j}")
                    rr_ap = rr[:]
                    rr_b = bass.AP(
                        rr_ap.tensor, rr_ap.offset,
                        [list(rr_ap.ap[0]), [0, HD], list(rr_ap.ap[-1])],
                    )
                    nc.scalar.dma_start(rb[:], rr_b)
                    nc.vector.scalar_tensor_tensor(
                        otd[j * HD:(j + 1) * HD, :],
                        ocs[j][0:HD, :], 0.0, rb[:],
                        op0=mybir.AluOpType.bypass, op1=mybir.AluOpType.mult,
                    )
                ot.append(otd)

            for d in range(H // 2):
                c, dd = duo_chunk[d]           # chunk, pair-in-chunk
                nduo = CHUNKS[c]
                cbase = 4 * CH_OFF[c] * DUO_ELEMS
                csz = 4 * nduo * DUO_ELEMS
                blk = kv_ag[cbase:cbase + csz]
                ktp = ktp_pool.tile([128, N], BF16, tag="ktp", name=f"ktp{d}")
                kt_view = blk.rearrange("(r x p q) -> x p r q",
                                        r=4, x=nduo * DUO_ELEMS // (128 * NQ),
                                        p=128, q=NQ)[dd]
                vaug = [
                    vsb_pool.tile([128, KT * (HD + 1)], BF16, tag="vsb",
                                  name=f"va{d}_{j}")
                    for j in range(2)
                ]
                # rank-major load order: rank r's K columns + V k-tiles land
                # together so kt = 4r can start before later ranks arrive
                for r in range(4):
                    nc.sync.dma_start(
                        ktp[:, r * NQ:(r + 1) * NQ], kt_view[:, r, :]
                    )
                    for j in range(2):
                        va4 = vaug[j][:].rearrange(
                            "cc (rr sh ef) -> cc rr sh ef", rr=4, sh=4, ef=HD + 1
                        )
                        vsrc = blk[
                            r * nduo * DUO_ELEMS + nduo * 128 * NQ:
                            (r + 1) * nduo * DUO_ELEMS
                        ].rearrange(
                            "(m p h e) -> p m h e", m=4, p=128, h=2 * nduo, e=HD
                        )[:, :, 2 * dd + j, :]
                        nc.sync.dma_start(va4[:, r, :, 0:HD], vsrc)
                for j in range(2):
                    va3 = vaug[j][:].rearrange("cc (t ef) -> cc t ef", ef=HD + 1)
                    nc.vector.memset(va3[:, :, HD:HD + 1], 1.0)

                # normalize the previous duo now that this duo's loads are queued
                if pend:
                    normalize(*pend.pop())

                o_acc = [
                    ps1.tile([128, NQ], FP32, tag="acc", name=f"oacc{d}_{j}")
                    for j in range(2)
                ]
                es_tiles = [None] * KT

                def emit_pv(kt_i):
                    es_kt = es_tiles[kt_i]
                    for j in range(2):
                        va3 = vaug[j][:].rearrange("cc (t ef) -> cc t ef", ef=HD + 1)
                        nc.tensor.matmul(
                            o_acc[j][0:HD + 1, :],
                            va3[:, kt_i, :],
                            es_kt[:, j * NQ:(j + 1) * NQ],
                            start=(kt_i == 0), stop=(kt_i == KT - 1),
                        )

                for kt in range(KT):
                    s = psS.tile([128, 2 * NQ], FP32, tag="squad", name=f"s{d}_{kt}")
                    for i in range(2):
                        nc.tensor.matmul(
                            s[:, i * NQ:(i + 1) * NQ],
                            ktp[i * HD:(i + 1) * HD, kt * 128:(kt + 1) * 128],
                            qt[d][i * HD:(i + 1) * HD, :],
                            start=True, stop=True,
                        )
                    es = es_pool.tile([128, 2 * NQ], BF16, tag="es",
                                      name=f"es{d}_{kt}")
                    nc.scalar.activation(
                        es[:], s[:], mybir.ActivationFunctionType.Exp, scale=SCALE
                    )
                    es_tiles[kt] = es
                    if kt >= 1:
                        emit_pv(kt - 1)
                emit_pv(KT - 1)
                pend.append((drain_oacc(o_acc, d), d))
            normalize(*pend.pop())

            # ---- output projection ----
            for m in range(NQ // 128):
                for n in range(D // 512):
                    ps = ps1.tile([128, 512], FP32, tag="acc", name=f"psP{m}{n}")
                    for k in range(CT):
                        nc.tensor.matmul(
                            ps[:], ot[k][:, m * 128:(m + 1) * 128],
                            wp[k][:, n * 512:(n + 1) * 512],
                            start=(k == 0), stop=(k == CT - 1),
                        )
                    y = y_pool.tile([128, 512], FP32, tag="yy", name=f"y{m}{n}")
                    nc.vector.scalar_tensor_tensor(
                        y[:], ps[:], 0.0, bp_sb[:, n * 512:(n + 1) * 512],
                        op0=mybir.AluOpType.bypass, op1=mybir.AluOpType.add,
                    )
                    nc.sync.dma_start(
                        out.ap()[m * 128:(m + 1) * 128, n * 512:(n + 1) * 512], y[:]
                    )

    nc.compile()
    return nc


def make_in_maps(x, W_qkv, b_qkv, W_proj, b_proj):
    x = np.asarray(x, dtype=np.float32)
    W_qkv = np.asarray(W_qkv, dtype=np.float32)
    b_qkv = np.asarray(b_qkv, dtype=np.float32)
    W_proj = np.asarray(W_proj, dtype=np.float32)
    b_proj = np.asarray(b_proj, dtype=np.float32)

    wq_bf = W_qkv.astype(ml_dtypes.bfloat16)
    wp_bf = W_proj.astype(ml_dtypes.bfloat16)
    bqk = np.ascontiguousarray(b_qkv[:2 * D].reshape(16, 128).T)
    bv = np.tile(b_qkv[2 * D:], (128, 1)).astype(np.float32)
    bp = np.tile(b_proj, (128, 1)).astype(np.float32)

    in_maps = []
    for c in range(NC):
        b, g = divmod(c, 4)
        xs = x[b, g * NQ:(g + 1) * NQ, :]
        in_maps.append({
            "xT": np.ascontiguousarray(xs.T).astype(ml_dtypes.bfloat16),
            "w_qkv": wq_bf,
            "w_proj": wp_bf,
            "b_qk": bqk,
            "b_v": bv,
            "b_prj": bp,
        })
    return in_maps


def run(inputs, trace=False):
    global _compiled
    if _compiled is None:
        _compiled = build()
    in_maps = make_in_maps(**inputs)
    res = bass_utils.run_bass_kernel_spmd(
        _compiled, in_maps, core_ids=list(range(NC)), trace=trace
    )
    full = np.empty((B, N, D), dtype=np.float32)
    for c in range(NC):
        b, g = divmod(c, 4)
        full[b, g * NQ:(g + 1) * NQ, :] = res.results[c]["out"]
    return full, res


def kernel(x, W_qkv, b_qkv, W_proj, b_proj):
    full, _ = run(dict(x=x, W_qkv=W_qkv, b_qkv=b_qkv, W_proj=W_proj, b_proj=b_proj))
    return full
